# revision 7
# baseline (speedup 1.0000x reference)
# Trainium2 Bass kernel for nn_BlockAttention_64141041598498.
#
# Module: x -> LN(full) -> depthwise3x3 -> window split -> SAGAN attention over
# N=4096 tokens -> combine(scramble) + residual -> LN(full, channels-last) ->
# gMLP (LN -> W1+gelu -> SGU with spatial proj Wsp[4096,4096] -> W2) + residual.
#
# Sharding: 8 cores = 4 batch samples x 2 SGU-feature-halves. Each core runs
# the full attention for its sample (duplicated within the pair); the gMLP
# spatial projection + W2 are split over the 512 SGU features (256 per core,
# host reorders W1's vg columns so each core's features are columns 0:256).
# W2 outputs are partial sums combined on the host. No cross-core comms.
import sys
import types
import contextlib
import ctypes

if "/opt/trn_rl_repo" not in sys.path:
    sys.path.insert(0, "/opt/trn_rl_repo")

import numpy as np
import concourse.bass as bass
import concourse.mybir as mybir
import concourse.tile as tile
from concourse.bass_utils import run_bass_kernel_spmd
from concourse.masks import make_identity

dt = mybir.dt
Alu = mybir.AluOpType
Act = mybir.ActivationFunctionType

# ---------------------------------------------------------------------------
# Environment patches (Tile emits sync patterns this walrus build rejects).
_MAX_WAITS = 1


def _patched_drain_and_barrier(self, tick_clock, wait_clock):
    from concourse.tile import ScopedClock

    nc = self.nc
    drain_inst = nc.sync.drain()
    wait_clock.add_sem_waits(
        drain_inst.ins, ScopedClock({None: tick_clock.global_clock})
    )
    si = drain_inst.ins.sync_info
    waits = list(si.on_wait) if si and si.on_wait else []
    if len(waits) > _MAX_WAITS:
        si.on_wait = waits[:_MAX_WAITS]
        rest = waits[_MAX_WAITS:]
        for i in range(0, len(rest), _MAX_WAITS):
            nop = nc.sync.nop(nofuse=True)
            nop.ins.sync_info = mybir.SyncInfo(
                on_wait=rest[i : i + _MAX_WAITS], on_update=[]
            )
    nc.all_engine_barrier()
    popped = nc._tile_sem_poison_stack.pop()
    assert popped is self._sem_poison
    nc.clear_and_free_semaphores(list(self.sems.allocated().values()))
    nc.all_engine_barrier()


tile.TileContext._drain_and_barrier = _patched_drain_and_barrier


def split_sync_waits(nc, max_waits=_MAX_WAITS):
    for fn in nc.m.functions:
        for bb in fn.blocks:
            insts = bb.instructions
            overloaded = []
            for idx, inst in enumerate(insts):
                si = inst.sync_info
                waits = list(si.on_wait) if si and si.on_wait else []
                if len(waits) > max_waits:
                    overloaded.append((idx, inst, waits))
            for idx, inst, waits in reversed(overloaded):
                excess = waits[: len(waits) - max_waits]
                inst.sync_info.on_wait = waits[len(waits) - max_waits :]
                nops = []
                for i in range(0, len(excess), max_waits):
                    eng = nc.engines[inst.engine]
                    nop = eng.nop(nofuse=True)
                    cur_list = nc.cur_bb.bb.instructions
                    assert cur_list and cur_list[-1].name == nop.ins.name
                    cur_list.pop()
                    nop.ins.sync_info = mybir.SyncInfo(
                        on_wait=excess[i : i + max_waits], on_update=[]
                    )
                    nops.append(nop.ins)
                for k, nop_inst in enumerate(nops):
                    insts.insert(idx + k, nop_inst)


def _ntff_profile_via_ctypes(so_path):
    try:
        lib = ctypes.CDLL(so_path)
    except OSError:
        return None
    if not hasattr(lib, "axon_start_nrt_profile"):
        return None
    lib.axon_start_nrt_profile.argtypes = [
        ctypes.POINTER(ctypes.c_int64),
        ctypes.c_size_t,
    ]
    lib.axon_start_nrt_profile.restype = ctypes.c_int64
    lib.axon_stop_nrt_profile.argtypes = [ctypes.c_char_p]
    lib.axon_stop_nrt_profile.restype = ctypes.c_int64

    @contextlib.contextmanager
    def _hook(output_dir, device_ids):
        import jax

        jax.devices()
        if device_ids:
            ids = (ctypes.c_int64 * len(device_ids))(*device_ids)
            rc = lib.axon_start_nrt_profile(ids, len(device_ids))
        else:
            rc = lib.axon_start_nrt_profile(None, 0)
        if rc != 0:
            raise RuntimeError(f"axon_start_nrt_profile rc={rc}")
        try:
            yield
        finally:
            n = lib.axon_stop_nrt_profile(str(output_dir).encode())
            print(f"ntff profile: {n} file(s) in {output_dir}", file=sys.stderr)

    return _hook


if "antenv.axon_hooks" not in sys.modules:
    _mod = types.ModuleType("antenv.axon_hooks")
    _the_hook = _ntff_profile_via_ctypes("/opt/axon/libaxon_pjrt.so")
    _mod.get_axon_ntff_profile_hook = lambda: _the_hook
    _mod.set_axon_ntff_profile_hook = lambda h: None
    sys.modules["antenv.axon_hooks"] = _mod

# ---------------------------------------------------------------------------
# Problem constants
B, C, H, W, WS = 4, 128, 64, 64, 8
C8 = C // 8
N = H * W            # 4096 tokens
DFF = 4 * C          # 512
DH = DFF // 2        # 256 SGU features per core
Hn, Wn = H // WS, W // WS
L = Hn * Wn
EPS = 1e-5
NT = N // 128        # 32 token tiles
NCH = N // 512       # 8 chunks

DEBUG = True
F32, F32R, BF16 = dt.float32, dt.float32r, dt.bfloat16


def build_bass():
    nc = bass.Bass()

    def inp(name, shape, d=F32):
        return nc.declare_dram_parameter(name, list(shape), d, isOutput=False)

    x_l = inp("x_l", [C, N])
    ln1g = inp("ln1g", [C, N])
    ln1b = inp("ln1b", [C, N])
    dww = inp("dww", [C, 9])
    dwb = inp("dwb", [C, 1])
    A_T = inp("A_T", [C, C])
    avec = inp("avec", [C, 1])
    vwT = inp("vwT", [C, C])
    vb_b = inp("vb_b", [C, C])
    gam = inp("gam", [1, 1])
    ln2gT = inp("ln2gT", [C, N])
    ln2bT = inp("ln2bT", [C, N])
    glng = inp("glng", [C, 1])
    glnb = inp("glnb", [C, 1])
    W1u = inp("W1u", [C, DH])         # u columns for this core's half
    W1v = inp("W1v", [C, DFF])        # vg columns, core-specific order:
    b1u = inp("b1u", [1, DH])         #   cols [0:256) = this core's half
    b1v = inp("b1v", [1, DFF])
    sgg_h = inp("sgg_h", [C, DH])     # sgu_g for this core's half features
    sbr_h = inp("sbr_h", [1, DH], BF16)   # sgu_b half row (rank-1 fold)
    wspT = inp("wspT", [N, N], BF16)      # Wsp.T full
    wsrow = inp("wsrow", [1, N], BF16)    # Wsp row sums
    bspt = inp("bspt", [C, NT])           # bsp rearranged [p, tile]
    W2h = inp("W2h", [DH, C])             # W2 rows for this core's half
    b2c = inp("b2c", [C, 1])
    res_c = inp("res_c", [C, 1])          # 1.0 on even cores, 0.0 on odd

    y_out = nc.declare_dram_parameter("y", [C, N], F32, isOutput=True)
    dbg = {}
    if DEBUG:
        for nm, shape in [
            ("d_wwin", [C, N]), ("d_kq", [C, N]), ("d_alpha", [C, 32]),
            ("d_h2", [C, N]), ("d_h2n", [C, N]), ("d_znaff", [C, N]),
            ("d_vgn", [C, DH]), ("d_u", [C, DH]), ("d_sp", [C, DH]),
            ("d_vt", [C, C]), ("d_exp", [C, 512]), ("d_sums", [1, N]),
        ]:
            dbg[nm] = nc.declare_dram_parameter(nm, shape, F32, isOutput=True)

    with tile.TileContext(nc) as tc, contextlib.ExitStack() as ctx:
        big = ctx.enter_context(tc.tile_pool(name="big", bufs=1))
        consts = ctx.enter_context(tc.tile_pool(name="consts", bufs=1))

        x_sb = big.tile([C, N], F32, tag="slotA")          # x, later u
        w_win = big.tile([C, N], F32R, tag="slotB")        # w, later znaff
        kq_sb = big.tile([C, N], F32R, tag="slotC")        # kq, later vgn(4MB)
        vT_sb = big.tile([C, NT, C], BF16, tag="slotD")
        h2_sb = big.tile([C, N], F32, tag="slotE")
        h2n = big.tile([C, N], F32R, tag="slotF")

        nc.sync.dma_start(out=x_sb, in_=x_l[:])

        ones_col = consts.tile([C, 1], F32)
        nc.vector.memset(ones_col, 1.0)
        ones_col_r = consts.tile([C, 1], F32R)
        nc.vector.tensor_copy(ones_col_r, ones_col)
        oo128_col = consts.tile([C, 1], F32)
        nc.vector.memset(oo128_col, 1.0 / 128.0)
        oo128_col_r = consts.tile([C, 1], F32R)
        nc.vector.tensor_copy(oo128_col_r, oo128_col)
        ones_row = consts.tile([1, C], F32)
        nc.vector.memset(ones_row, 1.0)
        ones_row_r = consts.tile([1, C], F32R)
        nc.vector.tensor_copy(ones_row_r, ones_row)
        eps_col = consts.tile([C, 1], F32)
        nc.vector.memset(eps_col, EPS)
        ident1 = consts.tile([1, 1], F32)
        nc.vector.memset(ident1, 1.0)
        ident128 = consts.tile([C, C], F32)
        make_identity(nc, ident128)
        gam_sb = consts.tile([1, 1], F32)
        nc.sync.dma_start(out=gam_sb, in_=gam[:])
        glng_sb = consts.tile([C, 1], F32)
        nc.sync.dma_start(out=glng_sb, in_=glng[:])
        glnb_sb = consts.tile([C, 1], F32)
        nc.sync.dma_start(out=glnb_sb, in_=glnb[:])
        b2c_sb = consts.tile([C, 1], F32)
        nc.sync.dma_start(out=b2c_sb, in_=b2c[:])
        res_sb = consts.tile([C, 1], F32)
        nc.sync.dma_start(out=res_sb, in_=res_c[:])
        bspt_sb = consts.tile([C, NT], F32)
        nc.sync.dma_start(out=bspt_sb, in_=bspt[:])
        alpha_t = consts.tile([C, 32], F32)

        def full_ln_stats(pool, psum_pool, src, scratch_tag):
            """mean/rstd of the whole [C, N] tensor -> [C,2] bcast cols."""
            stats2 = pool.tile([C, 2], F32, tag="st2")
            nc.vector.tensor_reduce(out=stats2[:, 0:1], in_=src.bitcast(F32),
                                    axis=mybir.AxisListType.X, op=Alu.add)
            sq_scr = pool.tile([C, N], F32, tag=scratch_tag)
            nc.scalar.activation(out=sq_scr, in_=src.bitcast(F32),
                                 func=Act.Square, accum_out=stats2[:, 1:2])
            st_ps = psum_pool.tile([1, 2], F32, tag="stps")
            nc.tensor.matmul(st_ps, ones_col, stats2, start=True, stop=True)
            SC = pool.tile([1, 8], F32, tag="SC")
            nc.vector.tensor_copy(SC[:, 0:2], st_ps)
            nc.vector.tensor_scalar_mul(SC[:, 2:3], SC[:, 0:1], 1.0 / (C * N))
            nc.vector.tensor_scalar_mul(SC[:, 4:5], SC[:, 1:2], 1.0 / (C * N))
            nc.vector.tensor_tensor(out=SC[:, 6:7], in0=SC[:, 2:3],
                                    in1=SC[:, 2:3], op=Alu.mult)
            nc.vector.tensor_tensor(out=SC[:, 7:8], in0=SC[:, 4:5],
                                    in1=SC[:, 6:7], op=Alu.subtract)
            sd1 = pool.tile([1, 1], F32, tag="sd1")
            nc.scalar.activation(out=sd1, in_=SC[:, 7:8], func=Act.Sqrt,
                                 bias=eps_col[0:1, :], scale=1.0)
            nc.vector.reciprocal(out=SC[:, 3:4], in_=sd1)
            ms_ps = psum_pool.tile([C, 2], F32, tag="msps")
            nc.tensor.matmul(ms_ps, ones_row, SC[:, 2:4], start=True,
                             stop=True)
            mr = pool.tile([C, 2], F32, tag="mr")
            nc.vector.tensor_copy(mr, ms_ps)
            return mr

        # =========== Phase 1: LN1 + depthwise conv + window permute =======
        with tc.tile_pool(name="p1", bufs=1) as p1, \
             tc.tile_pool(name="p1ps", bufs=1, space="PSUM") as p1ps:
            g1 = p1.tile([C, N], F32, tag="bigA")
            nc.sync.dma_start(out=g1, in_=ln1g[:])
            b1t = p1.tile([C, N], F32, tag="bigB")
            nc.sync.dma_start(out=b1t, in_=ln1b[:])
            dww_sb = p1.tile([C, 9], F32)
            nc.sync.dma_start(out=dww_sb, in_=dww[:])
            dwb_sb = p1.tile([C, 1], F32)
            nc.sync.dma_start(out=dwb_sb, in_=dwb[:])

            mr = full_ln_stats(p1, p1ps, x_sb, "bigC")
            xn = p1.tile([C, N], F32, tag="bigC")
            nc.vector.tensor_scalar(out=xn, in0=x_sb, scalar1=mr[:, 0:1],
                                    scalar2=mr[:, 1:2], op0=Alu.subtract,
                                    op1=Alu.mult)
            nc.vector.tensor_tensor(out=xn, in0=xn, in1=g1, op=Alu.mult)
            nc.vector.tensor_tensor(out=xn, in0=xn, in1=b1t, op=Alu.add)

            conv_sb = p1.tile([C, H, W], F32, tag="bigA")
            xn3 = xn.rearrange("c (h w) -> c h w", h=H)
            nc.vector.tensor_scalar(out=conv_sb, in0=xn3,
                                    scalar1=dww_sb[:, 4:5],
                                    scalar2=dwb_sb[:, 0:1],
                                    op0=Alu.mult, op1=Alu.add)
            for di in (-1, 0, 1):
                for dj in (-1, 0, 1):
                    if di == 0 and dj == 0:
                        continue
                    tap = (di + 1) * 3 + (dj + 1)
                    h0, h1 = max(0, -di), H - max(0, di)
                    w0, w1 = max(0, -dj), W - max(0, dj)
                    nc.vector.scalar_tensor_tensor(
                        out=conv_sb[:, h0:h1, w0:w1],
                        in0=xn3[:, h0 + di : h1 + di, w0 + dj : w1 + dj],
                        scalar=dww_sb[:, tap : tap + 1],
                        in1=conv_sb[:, h0:h1, w0:w1],
                        op0=Alu.mult, op1=Alu.add)

            conv_w = conv_sb.rearrange("c (hn i) (wn j) -> c hn wn i j",
                                       i=WS, j=WS)
            nc.vector.tensor_copy(
                w_win.rearrange("c (hn wn i j) -> c hn wn i j",
                                hn=Hn, wn=Wn, i=WS), conv_w)
            if DEBUG:
                wdbg = p1.tile([C, N], F32, tag="bigB")
                nc.vector.tensor_copy(wdbg, w_win.bitcast(F32))
                nc.sync.dma_start(out=dbg["d_wwin"][:], in_=wdbg)

        # =========== Phase 2: projections kq, vT, alpha ===================
        with tc.tile_pool(name="p2", bufs=1) as p2, \
             tc.tile_pool(name="p2ps", bufs=2, space="PSUM") as p2ps, \
             tc.tile_pool(name="p2ps1", bufs=1, space="PSUM") as p2ps1:
            AT_f = p2.tile([C, C], F32)
            nc.sync.dma_start(out=AT_f, in_=A_T[:])
            AT_sb = p2.tile([C, C], F32R)
            nc.vector.tensor_copy(AT_sb, AT_f)
            av_f = p2.tile([C, 1], F32)
            nc.sync.dma_start(out=av_f, in_=avec[:])
            av_sb = p2.tile([C, 1], F32R)
            nc.vector.tensor_copy(av_sb, av_f)
            vwT_f = p2.tile([C, C], F32)
            nc.sync.dma_start(out=vwT_f, in_=vwT[:])
            vwT_sb = p2.tile([C, C], F32R)
            nc.vector.tensor_copy(vwT_sb, vwT_f)
            vb_sb = p2.tile([C, C], F32)
            nc.sync.dma_start(out=vb_sb, in_=vb_b[:])

            alpha_row = p2.tile([1, N], F32)
            for j in range(NCH):
                sl = slice(512 * j, 512 * (j + 1))
                kq_ps = p2ps.tile([C, 512], F32, tag="kqps")
                nc.tensor.matmul(kq_ps, AT_sb, w_win[:, sl],
                                 start=True, stop=True)
                nc.vector.tensor_copy(kq_sb[:, sl], kq_ps)
                ar_ps = p2ps1.tile([1, 512], F32, tag="arps")
                nc.tensor.matmul(ar_ps, av_sb, w_win[:, sl],
                                 start=True, stop=True)
                nc.vector.tensor_copy(alpha_row[:, sl], ar_ps)
            for t in range(NT):
                msl = slice(128 * t, 128 * (t + 1))
                vt_ps = p2ps.tile([C, C], F32, tag="vtps")
                nc.tensor.matmul(vt_ps, w_win[:, msl], vwT_sb,
                                 start=True, stop=True)
                nc.vector.scalar_tensor_tensor(
                    out=vT_sb[:, t, :], in0=vt_ps, scalar=1.0,
                    in1=vb_sb, op0=Alu.mult, op1=Alu.add)
            al_ps = p2ps1.tile([C, 32], F32, tag="alps")
            for t in range(32):
                nc.tensor.transpose(al_ps[:, t : t + 1],
                                    alpha_row[0:1, 128 * t : 128 * (t + 1)],
                                    ident1)
            nc.vector.tensor_copy(alpha_t, al_ps)
            if DEBUG:
                nc.sync.dma_start(out=dbg["d_alpha"][:], in_=alpha_t)
                kqd = p2.tile([C, N], F32, tag="bigA")
                nc.vector.tensor_copy(kqd, kq_sb.bitcast(F32))
                nc.sync.dma_start(out=dbg["d_kq"][:], in_=kqd)
                vtd = p2.tile([C, C], F32)
                nc.vector.tensor_copy(vtd, vT_sb[:, 0, :])
                nc.sync.dma_start(out=dbg["d_vt"][:], in_=vtd)

        # =========== Phase 3: attention ===================================
        with tc.tile_pool(name="p3", bufs=2) as p3, \
             tc.tile_pool(name="p3T", bufs=2) as p3T, \
             tc.tile_pool(name="p3e", bufs=3) as p3e, \
             tc.tile_pool(name="ps_e", bufs=2, space="PSUM") as ps_e, \
             tc.tile_pool(name="ps_av", bufs=2, space="PSUM") as ps_av, \
             tc.tile_pool(name="ps_s", bufs=1, space="PSUM") as ps_s:
            if DEBUG:
                sums_dbg = p3.tile([1, N], F32, tag="sumsdbg")
            for j in range(NCH):
                sl = slice(512 * j, 512 * (j + 1))
                av_ps = ps_av.tile([C, 512], F32, tag="avps")
                T_j = p3T.tile([C, 512], F32R, tag="Tj")
                for i in range(NT):
                    e_ps = ps_e.tile([C, 512], F32, tag="eps")
                    nc.tensor.matmul(e_ps, kq_sb[:, 128 * i : 128 * (i + 1)],
                                     w_win[:, sl], start=True, stop=True)
                    ex = p3e.tile([C, 512], BF16, tag="ex")
                    nc.scalar.activation(out=ex, in_=e_ps, func=Act.Exp,
                                         bias=alpha_t[:, i : i + 1], scale=1.0)
                    if i == 0:
                        nc.vector.tensor_copy(T_j, ex)
                    else:
                        nc.vector.tensor_add(T_j, T_j, ex)
                    nc.tensor.matmul(av_ps, vT_sb[:, i, :], ex,
                                     start=(i == 0), stop=(i == NT - 1))
                    if DEBUG and j == 0 and i == 0:
                        exd = p3.tile([C, 512], F32, tag="exd")
                        nc.vector.tensor_copy(exd, ex)
                        nc.sync.dma_start(out=dbg["d_exp"][:], in_=exd)
                s_ps = ps_s.tile([1, 512], F32, tag="sps")
                nc.tensor.matmul(s_ps, ones_col_r, T_j, start=True, stop=True)
                r_row = p3.tile([1, 512], F32, tag="rrow")
                nc.vector.reciprocal(out=r_row, in_=s_ps)
                if DEBUG:
                    nc.vector.tensor_copy(sums_dbg[:, sl], s_ps)
                nc.vector.tensor_scalar_mul(r_row, r_row, gam_sb[:, 0:1])
                r_row_r = p3.tile([1, 512], F32R, tag="rrowr")
                nc.vector.tensor_copy(r_row_r, r_row)
                rb_ps = ps_s.tile([C, 512], F32, tag="rbps")
                nc.tensor.matmul(rb_ps, ones_row_r, r_row_r,
                                 start=True, stop=True)
                rb_sb = p3.tile([C, 512], F32, tag="rbsb")
                nc.vector.tensor_copy(rb_sb, rb_ps)
                t1 = p3.tile([C, 512], F32, tag="t1")
                nc.vector.tensor_tensor(out=t1, in0=av_ps, in1=rb_sb,
                                        op=Alu.mult)
                h2_str = bass.AP(
                    tensor=h2_sb.tensor,
                    offset=h2_sb.offset + 8 * j,
                    ap=[h2_sb.ap[0], [1, 8], [64, 64]])
                nc.vector.tensor_tensor(out=h2_str, in0=t1,
                                        in1=w_win[:, sl].bitcast(F32),
                                        op=Alu.add)
            nc.vector.tensor_tensor(out=h2_sb, in0=h2_sb, in1=x_sb,
                                    op=Alu.add)
            if DEBUG:
                nc.sync.dma_start(out=dbg["d_h2"][:], in_=h2_sb)
                nc.sync.dma_start(out=dbg["d_sums"][:], in_=sums_dbg)

        # =========== Phase 4: LN2 =========================================
        with tc.tile_pool(name="p4", bufs=1) as p4, \
             tc.tile_pool(name="p4ps", bufs=1, space="PSUM") as p4ps:
            g2 = p4.tile([C, N], F32, tag="bigA")
            nc.sync.dma_start(out=g2, in_=ln2gT[:])
            b2t = p4.tile([C, N], F32, tag="bigB")
            nc.sync.dma_start(out=b2t, in_=ln2bT[:])
            mr = full_ln_stats(p4, p4ps, h2_sb, "bigC")
            tmp = p4.tile([C, N], F32, tag="bigC")
            nc.vector.tensor_scalar(out=tmp, in0=h2_sb, scalar1=mr[:, 0:1],
                                    scalar2=mr[:, 1:2], op0=Alu.subtract,
                                    op1=Alu.mult)
            nc.vector.tensor_tensor(out=tmp, in0=tmp, in1=g2, op=Alu.mult)
            nc.vector.tensor_tensor(out=h2n, in0=tmp, in1=b2t, op=Alu.add)
            if DEBUG:
                h2nd = p4.tile([C, N], F32, tag="bigA")
                nc.vector.tensor_copy(h2nd, h2n.bitcast(F32))
                nc.sync.dma_start(out=dbg["d_h2n"][:], in_=h2nd)

        # =========== Phase 5: gln (per-token LN over C) + affine ==========
        znaff = big.tile([C, N], F32R, tag="slotB")  # w_win slot (dead)
        with tc.tile_pool(name="p5", bufs=1) as p5, \
             tc.tile_pool(name="p5ps", bufs=2, space="PSUM") as p5ps:
            mrow = p5.tile([1, N], F32R)
            qrow = p5.tile([1, N], F32R)
            for j in range(NCH):
                sl = slice(512 * j, 512 * (j + 1))
                m_ps = p5ps.tile([1, 512], F32, tag="mps")
                nc.tensor.matmul(m_ps, oo128_col_r, h2n[:, sl],
                                 start=True, stop=True)
                nc.vector.tensor_copy(mrow[:, sl], m_ps)
                sq = p5.tile([C, 512], F32, tag="sqc")
                nc.scalar.activation(out=sq, in_=h2n[:, sl].bitcast(F32),
                                     func=Act.Square)
                q_ps = p5ps.tile([1, 512], F32, tag="qps")
                nc.tensor.matmul(q_ps, oo128_col, sq, start=True, stop=True)
                nc.vector.tensor_copy(qrow[:, sl], q_ps)
            mean_b = p5.tile([C, N], F32, tag="bigA")
            var_b = p5.tile([C, N], F32, tag="bigB")
            for j in range(NCH):
                sl = slice(512 * j, 512 * (j + 1))
                mb_ps = p5ps.tile([C, 512], F32, tag="mbps")
                nc.tensor.matmul(mb_ps, ones_row_r, mrow[:, sl],
                                 start=True, stop=True)
                nc.vector.tensor_copy(mean_b[:, sl], mb_ps)
                qb_ps = p5ps.tile([C, 512], F32, tag="qbps")
                nc.tensor.matmul(qb_ps, ones_row_r, qrow[:, sl],
                                 start=True, stop=True)
                nc.vector.tensor_tensor(out=var_b[:, sl], in0=mean_b[:, sl],
                                        in1=mean_b[:, sl], op=Alu.mult)
                nc.vector.tensor_tensor(out=var_b[:, sl], in0=qb_ps,
                                        in1=var_b[:, sl], op=Alu.subtract)
            nc.scalar.activation(out=var_b, in_=var_b, func=Act.Sqrt,
                                 bias=eps_col, scale=1.0)
            nc.vector.reciprocal(out=var_b, in_=var_b)
            nc.vector.tensor_tensor(out=mean_b, in0=h2n.bitcast(F32),
                                    in1=mean_b, op=Alu.subtract)
            nc.vector.tensor_tensor(out=mean_b, in0=mean_b, in1=var_b,
                                    op=Alu.mult)
            nc.vector.tensor_scalar(out=znaff, in0=mean_b,
                                    scalar1=glng_sb[:, 0:1],
                                    scalar2=glnb_sb[:, 0:1],
                                    op0=Alu.mult, op1=Alu.add)
            if DEBUG:
                znd = p5.tile([C, N], F32, tag="bigA")
                nc.vector.tensor_copy(znd, znaff.bitcast(F32))
                nc.sync.dma_start(out=dbg["d_znaff"][:], in_=znd)

        # =========== Phase 6: W1 + gelu + SGU-LN; u and vgn ==============
        vgn_sb = big.tile([C, NT, DH], BF16, tag="slotC")  # kq slot (dead)
        u_sb = big.tile([C, NT, DH], BF16, tag="slotA")    # x slot (dead)
        with tc.tile_pool(name="p6", bufs=2) as p6, \
             tc.tile_pool(name="p6c", bufs=1) as p6c, \
             tc.tile_pool(name="p6ps", bufs=2, space="PSUM") as p6ps:
            W1v_f = p6c.tile([C, DFF], F32)
            nc.sync.dma_start(out=W1v_f, in_=W1v[:])
            W1v_sb = p6c.tile([C, DFF], F32R)
            nc.vector.tensor_copy(W1v_sb, W1v_f)
            W1u_f = p6c.tile([C, DH], F32)
            nc.sync.dma_start(out=W1u_f, in_=W1u[:])
            W1u_sb = p6c.tile([C, DH], F32R)
            nc.vector.tensor_copy(W1u_sb, W1u_f)
            b1v_f = p6c.tile([1, DFF], F32)
            nc.sync.dma_start(out=b1v_f, in_=b1v[:])
            b1v_sb = p6c.tile([1, DFF], F32R)
            nc.vector.tensor_copy(b1v_sb, b1v_f)
            b1u_f = p6c.tile([1, DH], F32)
            nc.sync.dma_start(out=b1u_f, in_=b1u[:])
            b1u_sb = p6c.tile([1, DH], F32R)
            nc.vector.tensor_copy(b1u_sb, b1u_f)
            sgg_sb = p6c.tile([C, DH], F32)
            nc.sync.dma_start(out=sgg_sb, in_=sgg_h[:])

            for t in range(NT):
                tsl = slice(128 * t, 128 * (t + 1))
                # vg: full 512 cols (for LN stats), normalize first 256
                vg_ps = p6ps.tile([C, DFF], F32, tag="vgps")
                nc.tensor.matmul(vg_ps, znaff[:, tsl], W1v_sb,
                                 start=True, stop=False)
                nc.tensor.matmul(vg_ps, ones_row_r, b1v_sb,
                                 start=False, stop=True)
                geh = p6.tile([C, DFF], F32, tag="geh")
                nc.scalar.activation(out=geh, in_=vg_ps, func=Act.Gelu)
                stats6 = p6.tile([C, 6], F32, tag="st6")
                nc.vector.bn_stats(out=stats6, in_=geh)
                mv = p6.tile([C, 2], F32, tag="mv2")
                nc.vector.bn_aggr(out=mv, in_=stats6)
                sd = p6.tile([C, 1], F32, tag="sd")
                nc.scalar.activation(out=sd, in_=mv[:, 1:2], func=Act.Sqrt,
                                     bias=eps_col, scale=1.0)
                rstd = p6.tile([C, 1], F32, tag="rstd")
                nc.vector.reciprocal(out=rstd, in_=sd)
                zn = p6.tile([C, DH], F32, tag="zn")
                nc.vector.tensor_scalar(out=zn, in0=geh[:, :DH],
                                        scalar1=mv[:, 0:1],
                                        scalar2=rstd[:, 0:1],
                                        op0=Alu.subtract, op1=Alu.mult)
                nc.vector.tensor_tensor(out=vgn_sb[:, t, :], in0=zn,
                                        in1=sgg_sb, op=Alu.mult)
                # u: half features only
                u_ps = p6ps.tile([C, DH], F32, tag="ups")
                nc.tensor.matmul(u_ps, znaff[:, tsl], W1u_sb,
                                 start=True, stop=False)
                nc.tensor.matmul(u_ps, ones_row_r, b1u_sb,
                                 start=False, stop=True)
                nc.scalar.activation(out=u_sb[:, t, :], in_=u_ps,
                                     func=Act.Gelu)
            if DEBUG:
                vgd = p6.tile([C, DH], F32, tag="zn")
                nc.vector.tensor_copy(vgd, vgn_sb[:, 0, :])
                nc.sync.dma_start(out=dbg["d_vgn"][:], in_=vgd)
                ud = p6.tile([C, DH], F32, tag="zn")
                nc.vector.tensor_copy(ud, u_sb[:, 0, :])
                nc.sync.dma_start(out=dbg["d_u"][:], in_=ud)

        # =========== Phase 7: spatial proj + gate + W2 + residual ========
        with tc.tile_pool(name="p7w", bufs=2) as p7w, \
             tc.tile_pool(name="p7", bufs=2) as p7, \
             tc.tile_pool(name="p7c", bufs=1) as p7c, \
             tc.tile_pool(name="p7pt", bufs=2) as p7pt, \
             tc.tile_pool(name="ps_sp", bufs=2, space="PSUM") as ps_sp, \
             tc.tile_pool(name="ps_tp", bufs=2, space="PSUM") as ps_tp, \
             tc.tile_pool(name="ps_w2", bufs=2, space="PSUM") as ps_w2:
            W2_f = p7c.tile([C, 2, C], F32)
            nc.sync.dma_start(
                out=W2_f,
                in_=W2h.rearrange("(t p) c -> p t c", p=128))
            W2_sb = p7c.tile([C, 2, C], F32R)
            nc.vector.tensor_copy(W2_sb, W2_f)
            sbr_sb = p7c.tile([1, DH], BF16)
            nc.sync.dma_start(out=sbr_sb, in_=sbr_h[:])
            wsr_sb = p7c.tile([1, N], BF16)
            nc.sync.dma_start(out=wsr_sb, in_=wsrow[:])

            for cw in range(NCH):          # output token chunks of 512
                w2_ps = ps_w2.tile([C, 512], F32, tag="w2ps")
                PT = p7pt.tile([C, 2, 512], F32R, tag="PT")
                for tt in range(4):        # 4 token tiles per chunk
                    t = 4 * cw + tt
                    # spatial projection for token tile t (128 rows, DH cols)
                    sp_ps = ps_sp.tile([C, DH], F32, tag="spps")
                    wsp_t = p7w.tile([C, NT, 128], BF16, tag="wspt")
                    nc.sync.dma_start(
                        out=wsp_t,
                        in_=wspT.rearrange("(mt p) n -> p mt n", p=128)
                        [:, :, 128 * t : 128 * (t + 1)])
                    for mt in range(NT):
                        nc.tensor.matmul(sp_ps, wsp_t[:, mt, :],
                                         vgn_sb[:, mt, :],
                                         start=(mt == 0), stop=False)
                    nc.tensor.matmul(
                        sp_ps, wsr_sb[:, 128 * t : 128 * (t + 1)], sbr_sb,
                        start=False, stop=True)
                    P_t = p7.tile([C, DH], F32, tag="Pt")
                    nc.vector.scalar_tensor_tensor(
                        out=P_t, in0=sp_ps, scalar=bspt_sb[:, t : t + 1],
                        in1=u_sb[:, t, :], op0=Alu.add, op1=Alu.mult)
                    if DEBUG and t == 0:
                        spd = p7.tile([C, DH], F32, tag="spd")
                        nc.vector.scalar_tensor_tensor(
                            out=spd, in0=sp_ps,
                            scalar=bspt_sb[:, t : t + 1],
                            in1=u_sb[:, t, :], op0=Alu.add, op1=Alu.bypass)
                        nc.sync.dma_start(out=dbg["d_sp"][:], in_=spd)
                    for ff in range(2):    # transpose P -> PT
                        tp_ps = ps_tp.tile([C, C], F32, tag="tpps")
                        nc.tensor.transpose(
                            tp_ps, P_t[:, 128 * ff : 128 * (ff + 1)],
                            ident128)
                        nc.vector.tensor_copy(
                            PT[:, ff, 128 * tt : 128 * (tt + 1)], tp_ps)
                for ff in range(2):
                    nc.tensor.matmul(w2_ps, W2_sb[:, ff, :], PT[:, ff, :],
                                     start=(ff == 0), stop=(ff == 1))
                # y = w2_ps + res_c * (h2n + b2)
                csl = slice(512 * cw, 512 * (cw + 1))
                tmp = p7.tile([C, 512], F32, tag="tmpy")
                nc.vector.tensor_scalar(out=tmp, in0=h2n[:, csl].bitcast(F32),
                                        scalar1=b2c_sb[:, 0:1],
                                        scalar2=res_sb[:, 0:1],
                                        op0=Alu.add, op1=Alu.mult)
                y_sb = p7.tile([C, 512], F32, tag="ysb")
                nc.vector.tensor_tensor(out=y_sb, in0=w2_ps, in1=tmp,
                                        op=Alu.add)
                nc.sync.dma_start(out=y_out[:, csl], in_=y_sb)

    split_sync_waits(nc)
    return nc, dbg


_BASS_CACHE = {}


def get_bass():
    key = ("main", DEBUG)
    if key not in _BASS_CACHE:
        _BASS_CACHE[key] = build_bass()
    return _BASS_CACHE[key]


def prepare_inputs(inputs):
    """Host-side prep: slice/reorder full inputs into 8 per-core maps."""
    f32 = np.float32
    x = np.asarray(inputs["x"], f32)
    qw, kw = np.asarray(inputs["qw"], f32), np.asarray(inputs["kw"], f32)
    qb, kb = np.asarray(inputs["qb"], f32), np.asarray(inputs["kb"], f32)
    vw, vb = np.asarray(inputs["vw"], f32), np.asarray(inputs["vb"], f32)
    W1 = np.asarray(inputs["W1"], f32)
    b1 = np.asarray(inputs["b1"], f32)
    Wsp = np.asarray(inputs["Wsp"], f32)
    bsp = np.asarray(inputs["bsp"], f32)
    W2 = np.asarray(inputs["W2"], f32)
    b2 = np.asarray(inputs["b2"], f32)
    sgu_g = np.asarray(inputs["sgu_g"], f32)
    sgu_b = np.asarray(inputs["sgu_b"], f32)

    base = dict(
        ln1g=np.ascontiguousarray(
            np.asarray(inputs["ln1_g"], f32).reshape(C, N)),
        ln1b=np.ascontiguousarray(
            np.asarray(inputs["ln1_b"], f32).reshape(C, N)),
        dww=np.ascontiguousarray(
            np.asarray(inputs["dw_w"], f32).reshape(C, 9)),
        dwb=np.asarray(inputs["dw_b"], f32).reshape(C, 1),
        A_T=np.ascontiguousarray(kw.T @ qw),
        avec=np.ascontiguousarray((kw.T @ qb).reshape(C, 1)),
        vwT=np.ascontiguousarray(vw.T),
        vb_b=np.ascontiguousarray(np.tile(vb[None, :], (C, 1))),
        gam=np.asarray(inputs["att_gamma"], f32).reshape(1, 1),
        ln2gT=np.ascontiguousarray(
            np.asarray(inputs["ln2_g"], f32).reshape(N, C).T),
        ln2bT=np.ascontiguousarray(
            np.asarray(inputs["ln2_b"], f32).reshape(N, C).T),
        glng=np.asarray(inputs["gln_g"], f32).reshape(C, 1),
        glnb=np.asarray(inputs["gln_b"], f32).reshape(C, 1),
        bspt=np.ascontiguousarray(bsp.reshape(NT, 128).T),
        b2c=b2.reshape(C, 1),
    )
    import ml_dtypes
    bf16 = ml_dtypes.bfloat16
    base["wspT"] = np.ascontiguousarray(Wsp.T).astype(bf16)
    base["wsrow"] = Wsp.sum(axis=1)[None, :].astype(bf16)

    in_maps = []
    for core in range(8):
        b, r = core // 2, core % 2
        fh = slice(DH * r, DH * (r + 1))          # this core's vg features
        fo = slice(DH * (1 - r), DH * (2 - r))    # the other half
        W1v_re = np.concatenate([W1[:, DFF:][:, fh], W1[:, DFF:][:, fo]],
                                axis=1)
        b1v_re = np.concatenate([b1[DFF:][fh], b1[DFF:][fo]])[None, :]
        m = dict(base)
        m["x_l"] = np.ascontiguousarray(x[b].reshape(C, N))
        m["W1u"] = np.ascontiguousarray(W1[:, :DFF][:, fh])
        m["b1u"] = np.ascontiguousarray(b1[:DFF][fh][None, :].astype(f32))
        m["W1v"] = np.ascontiguousarray(W1v_re)
        m["b1v"] = np.ascontiguousarray(b1v_re.astype(f32))
        m["sgg_h"] = np.ascontiguousarray(
            np.tile(sgu_g[fh][None, :], (C, 1)))
        m["sbr_h"] = sgu_b[fh][None, :].astype(bf16)
        m["W2h"] = np.ascontiguousarray(W2[fh])
        m["res_c"] = np.full((C, 1), 1.0 - r, f32)
        in_maps.append(m)
    return in_maps


def assemble_output(results):
    y = np.zeros((B, C, N), np.float32)
    for core in range(8):
        b = core // 2
        y[b] += results[core]["y"]
    # y is [b, c, q]; output is (B, H, W, C) with q = h*64+w
    return np.ascontiguousarray(y.reshape(B, C, H, W).transpose(0, 2, 3, 1))


def kernel(_trace=False, _tmpdir=None, **inputs):
    nc, _ = get_bass()
    in_maps = prepare_inputs(inputs)
    kw = {}
    if _trace:
        kw = dict(trace=True, tmpdir=_tmpdir)
    res = run_bass_kernel_spmd(nc, in_maps, list(range(8)), **kw)
    out = assemble_output(res.results)
    kernel.last_exec_time_ns = res.exec_time_ns
    kernel.last_results = res.results
    return out


if __name__ == "__main__":
    print("kernel module OK")
